# revision 5
# baseline (speedup 1.0000x reference)
"""RBF kernel matrix on 8 Trainium2 NeuronCores.

K[i, j] = exp(-gamma * ||x_i - y_j||^2),  x: (8192, 64), y: (8192, 64).

Strategy: shard rows of x across the 8 cores (1024 rows each), replicate y.
On the host we fold the squared norms and gamma into augmented K=66 operands

    x~[:, i] = [x_i, ||x_i||^2, 1]          (66, 1024) per core
    y~[:, j] = [2g*y_j, -g, -g*||y_j||^2]   (66, 8192) replicated

so a single fp32 matmul  x~.T @ y~  produces -gamma*dist^2 directly in PSUM,
and one ScalarE Exp activation per PSUM group finishes the tile. Each core
writes its (1024, 8192) fp32 block straight to its DRAM output, which the
host concatenates.
"""

import numpy as np

from concourse import bacc, bass, tile, mybir
from concourse.bass_utils import run_bass_kernel_spmd

N_CORES = 8
BX, BY, F = 8192, 8192, 64
M_CORE = BX // N_CORES      # 1024 rows of x per core
KAUG = F + 2                # 66: features + |x|^2 slot + 1 slot
MM_N = 512                  # fp32 moving-operand max / one PSUM bank
GRP = 4                     # PSUM banks per activation group
GRP_N = MM_N * GRP          # 2048 columns per ACT/DMA tile

_cache: dict = {}


def _build():
    if "nc" in _cache:
        return _cache["nc"], _cache["names"]

    f32 = mybir.dt.float32
    nc = bacc.Bacc(None, target_bir_lowering=False, debug=False)
    xT = nc.dram_tensor("xT", (KAUG, M_CORE), f32, kind="ExternalInput")
    yT = nc.dram_tensor("yT", (KAUG, BY), f32, kind="ExternalInput")
    out = nc.dram_tensor("out", (M_CORE, BY), f32, kind="ExternalOutput")

    with tile.TileContext(nc) as tc:
        with (
            tc.tile_pool(name="const", bufs=1) as cpool,
            tc.tile_pool(name="obuf", bufs=4) as opool,
            tc.tile_pool(name="psum", bufs=2, space="PSUM") as ppool,
        ):
            xT_sb = cpool.tile((KAUG, M_CORE), f32)
            yT_sb = cpool.tile((KAUG, BY), f32)
            nc.sync.dma_start(out=xT_sb[:], in_=xT[:])
            nc.sync.dma_start(out=yT_sb[:], in_=yT[:])

            for mi in range(M_CORE // 128):          # 8 chunks of 128 rows
                lhsT = xT_sb[:, mi * 128 : (mi + 1) * 128]
                for ni in range(BY // GRP_N):        # 4 groups of 2048 cols
                    ps = ppool.tile((128, GRP_N), f32)
                    for j in range(GRP):
                        c0 = ni * GRP_N + j * MM_N
                        nc.tensor.matmul(
                            ps[:, j * MM_N : (j + 1) * MM_N],
                            lhsT,
                            yT_sb[:, c0 : c0 + MM_N],
                            start=True,
                            stop=True,
                        )
                    ot = opool.tile((128, GRP_N), f32)
                    nc.scalar.activation(
                        ot[:], ps[:], mybir.ActivationFunctionType.Exp
                    )
                    nc.sync.dma_start(
                        out=out[
                            mi * 128 : (mi + 1) * 128,
                            ni * GRP_N : (ni + 1) * GRP_N,
                        ],
                        in_=ot[:],
                    )

    nc.compile()
    _cache["nc"] = nc
    _cache["names"] = ("xT", "yT", "out")
    return nc, _cache["names"]


def _prep_inputs(x, y, gamma):
    x = np.ascontiguousarray(np.asarray(x, dtype=np.float32))
    y = np.ascontiguousarray(np.asarray(y, dtype=np.float32))
    g = float(np.asarray(gamma, dtype=np.float32))

    xa = np.empty((KAUG, BX), dtype=np.float32)
    xa[:F] = x.T
    xa[F] = (x * x).sum(axis=1)
    xa[F + 1] = 1.0

    ya = np.empty((KAUG, BY), dtype=np.float32)
    ya[:F] = (2.0 * g) * y.T
    ya[F] = -g
    ya[F + 1] = -g * (y * y).sum(axis=1)
    return xa, ya


def _run(x, y, gamma, trace=False, tmpdir=None):
    nc, (xn, yn, on) = _build()
    xa, ya = _prep_inputs(x, y, gamma)
    in_maps = [
        {xn: np.ascontiguousarray(xa[:, c * M_CORE : (c + 1) * M_CORE]), yn: ya}
        for c in range(N_CORES)
    ]
    res = run_bass_kernel_spmd(
        nc, in_maps, list(range(N_CORES)), trace=trace, tmpdir=tmpdir
    )
    full = np.concatenate([res.results[c][on] for c in range(N_CORES)], axis=0)
    return full, res


def kernel(x, y, gamma):
    full, _ = _run(x, y, gamma, trace=False)
    return full


def kernel_traced(x, y, gamma, tmpdir=None):
    """test.py helper: returns (output, BassKernelResults with profile)."""
    return _run(x, y, gamma, trace=True, tmpdir=tmpdir)


# revision 7
# speedup vs baseline: 1.9638x; 1.9638x over previous
"""RBF kernel matrix on 8 Trainium2 NeuronCores.

K[i, j] = exp(-gamma * ||x_i - y_j||^2),  x: (8192, 64), y: (8192, 64).

Strategy: shard rows of x across the 8 cores (1024 rows each), replicate y.
On the host we fold the squared norms and gamma into augmented K=66 operands

    x~[:, i] = [x_i, ||x_i||^2, 1]          (66, 1024) per core
    y~[:, j] = [2g*y_j, -g, -g*||y_j||^2]   (66, 8192) replicated

so a single fp32 matmul  x~.T @ y~  produces -gamma*dist^2 directly in PSUM,
and one ScalarE Exp activation per PSUM group finishes the tile. Each core
writes its (1024, 8192) fp32 block straight to its DRAM output, which the
host concatenates.
"""

import numpy as np

from concourse import bacc, bass, tile, mybir
from concourse.bass_utils import run_bass_kernel_spmd

N_CORES = 8
BX, BY, F = 8192, 8192, 64
M_CORE = BX // N_CORES      # 1024 rows of x per core
KAUG = F + 2                # 66: features + |x|^2 slot + 1 slot
MM_N = 512                  # fp32 moving-operand max / one PSUM bank
GRP = 4                     # PSUM banks per activation group
GRP_N = MM_N * GRP          # 2048 columns per ACT/DMA tile

_cache: dict = {}


def _build():
    if "nc" in _cache:
        return _cache["nc"], _cache["names"]

    f32 = mybir.dt.float32
    f32r_in = mybir.dt.float32r
    nc = bacc.Bacc(None, target_bir_lowering=False, debug=False)
    xT = nc.dram_tensor("xT", (KAUG, M_CORE), f32r_in, kind="ExternalInput")
    yT = nc.dram_tensor("yT", (KAUG, BY), f32r_in, kind="ExternalInput")
    out = nc.dram_tensor("out", (M_CORE, BY), f32, kind="ExternalOutput")

    with tile.TileContext(nc) as tc:
        with (
            tc.tile_pool(name="const", bufs=1) as cpool,
            tc.tile_pool(name="obuf", bufs=4) as opool,
            tc.tile_pool(name="psum", bufs=2, space="PSUM") as ppool,
        ):
            # float32r: same fp32 bits, but PE streams it at 1 cycle/row
            # (vs 4 for plain float32) when the moving dim is >=256.
            f32r = mybir.dt.float32r
            xT_sb = cpool.tile((KAUG, M_CORE), f32r)
            yT_sb = cpool.tile((KAUG, BY), f32r)
            nc.sync.dma_start(out=xT_sb[:], in_=xT[:])
            nc.sync.dma_start(out=yT_sb[:], in_=yT[:])

            for mi in range(M_CORE // 128):          # 8 chunks of 128 rows
                lhsT = xT_sb[:, mi * 128 : (mi + 1) * 128]
                for ni in range(BY // GRP_N):        # 4 groups of 2048 cols
                    ps = ppool.tile((128, GRP_N), f32)
                    for j in range(GRP):
                        c0 = ni * GRP_N + j * MM_N
                        nc.tensor.matmul(
                            ps[:, j * MM_N : (j + 1) * MM_N],
                            lhsT,
                            yT_sb[:, c0 : c0 + MM_N],
                            start=True,
                            stop=True,
                        )
                    ot = opool.tile((128, GRP_N), f32)
                    nc.scalar.activation(
                        ot[:], ps[:], mybir.ActivationFunctionType.Exp
                    )
                    nc.sync.dma_start(
                        out=out[
                            mi * 128 : (mi + 1) * 128,
                            ni * GRP_N : (ni + 1) * GRP_N,
                        ],
                        in_=ot[:],
                    )

    nc.compile()
    _cache["nc"] = nc
    _cache["names"] = ("xT", "yT", "out")
    return nc, _cache["names"]


def _prep_inputs(x, y, gamma):
    x = np.ascontiguousarray(np.asarray(x, dtype=np.float32))
    y = np.ascontiguousarray(np.asarray(y, dtype=np.float32))
    g = float(np.asarray(gamma, dtype=np.float32))

    xa = np.empty((KAUG, BX), dtype=np.float32)
    xa[:F] = x.T
    xa[F] = (x * x).sum(axis=1)
    xa[F + 1] = 1.0

    ya = np.empty((KAUG, BY), dtype=np.float32)
    ya[:F] = (2.0 * g) * y.T
    ya[F] = -g
    ya[F + 1] = -g * (y * y).sum(axis=1)
    return xa, ya


def _run(x, y, gamma, trace=False, tmpdir=None):
    nc, (xn, yn, on) = _build()
    xa, ya = _prep_inputs(x, y, gamma)
    in_maps = [
        {xn: np.ascontiguousarray(xa[:, c * M_CORE : (c + 1) * M_CORE]), yn: ya}
        for c in range(N_CORES)
    ]
    res = run_bass_kernel_spmd(
        nc, in_maps, list(range(N_CORES)), trace=trace, tmpdir=tmpdir
    )
    full = np.concatenate([res.results[c][on] for c in range(N_CORES)], axis=0)
    return full, res


def kernel(x, y, gamma):
    full, _ = _run(x, y, gamma, trace=False)
    return full


def kernel_traced(x, y, gamma, tmpdir=None):
    """test.py helper: returns (output, BassKernelResults with profile)."""
    return _run(x, y, gamma, trace=True, tmpdir=tmpdir)


# revision 8
# speedup vs baseline: 2.0868x; 1.0626x over previous
"""RBF kernel matrix on 8 Trainium2 NeuronCores.

K[i, j] = exp(-gamma * ||x_i - y_j||^2),  x: (8192, 64), y: (8192, 64).

Strategy: shard rows of x across the 8 cores (1024 rows each), replicate y.
On the host we express -gamma*dist^2 as a single inner product of augmented
vectors, then split each factor into an fp16 hi/lo pair (22-bit effective
mantissa) so the PE can run at its fast 1-cycle/row fp16 rate instead of the
4x-slower fp32 path. The three needed cross products (hi*hi, lo*hi, hi*lo)
are stacked along the contraction dim:

    rows   0..63   xh_k  * yh_k      (feature hi*hi)
    rows  64..127  xl_k  * yh_k      (feature lo*hi)
    rows 128..191  xh_k  * yl_k      (feature hi*lo)
    rows 192..194  |x|^2 hi/lo pairs against -gamma hi/lo
    rows 195..196  1 * (-gamma*|y|^2) hi/lo

giving 197 rows total = one K=128 matmul + one K=69 matmul accumulating
into the same PSUM bank. PSUM then holds -gamma*dist^2 in fp32 (all fp16
products are exact in fp32), one ScalarE Exp activation per 4-bank PSUM
group finishes the tile, and each core DMAs its (1024, 8192) fp32 block
out in 1 MiB chunks.
"""

import numpy as np

from concourse import bacc, tile, mybir
from concourse.bass_utils import run_bass_kernel_spmd

N_CORES = 8
BX, BY, F = 8192, 8192, 64
M_CORE = BX // N_CORES      # 1024 rows of x per core
K1 = 128                    # rows in the first stacked matmul
K2 = 69                     # rows in the second (197 - 128)
MM_N = 512                  # one PSUM bank of fp32
GRP = 4                     # PSUM banks per activation group
GRP_N = MM_N * GRP          # 2048 columns per ACT/DMA tile

_cache: dict = {}


def _build():
    if "nc" in _cache:
        return _cache["nc"]

    f32 = mybir.dt.float32
    f16 = mybir.dt.float16
    nc = bacc.Bacc(None, target_bir_lowering=False, debug=False)
    x1 = nc.dram_tensor("x1", (K1, M_CORE), f16, kind="ExternalInput")
    x2 = nc.dram_tensor("x2", (K2, M_CORE), f16, kind="ExternalInput")
    y1 = nc.dram_tensor("y1", (K1, BY), f16, kind="ExternalInput")
    y2 = nc.dram_tensor("y2", (K2, BY), f16, kind="ExternalInput")
    out = nc.dram_tensor("out", (M_CORE, BY), f32, kind="ExternalOutput")

    with tile.TileContext(nc) as tc:
        with (
            tc.tile_pool(name="const", bufs=1) as cpool,
            tc.tile_pool(name="obuf", bufs=4) as opool,
            tc.tile_pool(name="psum", bufs=2, space="PSUM") as ppool,
        ):
            x1_sb = cpool.tile((K1, M_CORE), f16)
            x2_sb = cpool.tile((K2, M_CORE), f16)
            y1_sb = cpool.tile((K1, BY), f16)
            y2_sb = cpool.tile((K2, BY), f16)
            nc.sync.dma_start(out=x1_sb[:], in_=x1[:])
            nc.sync.dma_start(out=x2_sb[:], in_=x2[:])
            nc.sync.dma_start(out=y1_sb[:], in_=y1[:])
            nc.sync.dma_start(out=y2_sb[:], in_=y2[:])

            for mi in range(M_CORE // 128):          # 8 chunks of 128 rows
                w1 = x1_sb[:, mi * 128 : (mi + 1) * 128]
                w2 = x2_sb[:, mi * 128 : (mi + 1) * 128]
                for ni in range(BY // GRP_N):        # 4 groups of 2048 cols
                    ps = ppool.tile((128, GRP_N), f32)
                    # weight-major order: 4 banks with W1, then 4 with W2,
                    # so the PE reloads weights twice per group, not 8x.
                    for j in range(GRP):
                        c0 = ni * GRP_N + j * MM_N
                        nc.tensor.matmul(
                            ps[:, j * MM_N : (j + 1) * MM_N],
                            w1,
                            y1_sb[:, c0 : c0 + MM_N],
                            start=True,
                            stop=False,
                        )
                    for j in range(GRP):
                        c0 = ni * GRP_N + j * MM_N
                        nc.tensor.matmul(
                            ps[:, j * MM_N : (j + 1) * MM_N],
                            w2,
                            y2_sb[:, c0 : c0 + MM_N],
                            start=False,
                            stop=True,
                        )
                    ot = opool.tile((128, GRP_N), f32)
                    nc.scalar.activation(
                        ot[:], ps[:], mybir.ActivationFunctionType.Exp
                    )
                    nc.sync.dma_start(
                        out=out[
                            mi * 128 : (mi + 1) * 128,
                            ni * GRP_N : (ni + 1) * GRP_N,
                        ],
                        in_=ot[:],
                    )

    nc.compile()
    _cache["nc"] = nc
    return nc


def _split16(a):
    hi = a.astype(np.float16)
    lo = (a - hi.astype(np.float32)).astype(np.float16)
    return hi, lo


def _prep_inputs(x, y, gamma):
    x = np.ascontiguousarray(np.asarray(x, dtype=np.float32))
    y = np.ascontiguousarray(np.asarray(y, dtype=np.float32))
    g = np.float32(np.asarray(gamma, dtype=np.float32))

    xh, xl = _split16(x.T)                    # (64, 8192) each
    x_sq = (x.astype(np.float64) ** 2).sum(axis=1).astype(np.float32)
    xsh, xsl = _split16(x_sq[None, :])        # (1, 8192)

    yt = y.T * (2.0 * g)                      # fold 2*gamma into y features
    yh, yl = _split16(yt)                     # (64, 8192)
    y_sq = (y.astype(np.float64) ** 2).sum(axis=1).astype(np.float32)
    yq = (-g) * y_sq[None, :]
    yqh, yql = _split16(yq)                   # (1, 8192)
    gh, gl = _split16(np.full((1, BY), -g, dtype=np.float32))

    ones = np.ones((1, BY), dtype=np.float16)

    # x-side stacked rows (197, 8192) and matching y-side rows
    xs = np.concatenate(
        [xh, xl, xh, xsh, xsl, xsh, ones, ones], axis=0
    )  # 64+64+64+1+1+1+1+1 = 197
    ys = np.concatenate(
        [yh, yh, yl, gh, gh, gl, yqh, yql], axis=0
    )
    xs1, xs2 = xs[:K1], xs[K1:]
    ys1, ys2 = ys[:K1], ys[K1:]
    return xs1, xs2, np.ascontiguousarray(ys1), np.ascontiguousarray(ys2)


def _run(x, y, gamma, trace=False, tmpdir=None):
    nc = _build()
    xs1, xs2, ys1, ys2 = _prep_inputs(x, y, gamma)
    in_maps = [
        {
            "x1": np.ascontiguousarray(xs1[:, c * M_CORE : (c + 1) * M_CORE]),
            "x2": np.ascontiguousarray(xs2[:, c * M_CORE : (c + 1) * M_CORE]),
            "y1": ys1,
            "y2": ys2,
        }
        for c in range(N_CORES)
    ]
    res = run_bass_kernel_spmd(
        nc, in_maps, list(range(N_CORES)), trace=trace, tmpdir=tmpdir
    )
    full = np.concatenate([res.results[c]["out"] for c in range(N_CORES)], axis=0)
    return full, res


def kernel(x, y, gamma):
    full, _ = _run(x, y, gamma, trace=False)
    return full


def kernel_traced(x, y, gamma, tmpdir=None):
    """test.py helper: returns (output, BassKernelResults with profile)."""
    return _run(x, y, gamma, trace=True, tmpdir=tmpdir)
